# revision 2
# baseline (speedup 1.0000x reference)
"""Multi-head attention + residual + LayerNorm on 8 Trainium2 NeuronCores, v6.

Sharding: core c handles batch b = c//4 and query-row quarter r = c%4
(rows 512r..512r+512 of S=2048) with ALL 16 heads.  K/V projected
redundantly per batch group (collectives ~110us/2MB on this stack),
projection work interleaved into the ACT-bound attention chunk stream.

v6 over v4 (mixed fp8/bf16 + DoubleRow where the math tolerates it):
- Q/K path stays bf16 (weights host-scaled x32): softmax exponentiates
  score errors, and fp8 Q/K measured 2.5e-2 rel err (over tolerance).
- V and O projections and the AV matmul run fp8e4 DoubleRow (2 k-tiles
  per instruction): V lands fp8 at x32 scale, ctx fp8 at x32, Wo fp8
  x32; the x1024 on the O-projection PSUM is matched by a 1024*I
  residual identity, and LayerNorm is scale-invariant (rstd absorbs
  it, eps 1e-5 is negligible at either scale for this data).
- exp runs with scale 0.125/1024 and bias -3.5 so et fits fp8e4: max
  scaled score on this input set is 8.29, and TRN fp8e4 becomes Inf
  above 240 (et_max = e^4.79 = 120, 2x margin; bias cancels in
  softmax).  AV consumes et over chunk pairs [P, 2, 2, SL].
- Softmax denominators: reciprocal of a [1,1024] row was 3.2us on DVE
  (big-tile reciprocal is ~20x slower than copy); now the row is
  DMA-reshaped to [128,8], reciprocal'd there (~0.2us), DMA'd back,
  then PE-broadcast (ones matmul) as before.
- Q/K projection PSUM->SBUF copies moved from ACT to DVE
  (tensor_scalar add with per-partition bias AP): ACT does (almost)
  nothing but the 128 exps at (1024+352)/1.2 = 1147ns each, which are
  the kernel's critical path and lower bound (~147us).
- Projection backlog is deadline-ordered and paced: K/Q items (1.7us
  each) drain every other chunk in pairs 1-3 and 5-6, V n=1 (0.85us)
  every chunk of pair 4, so PE never oversubscribes a chunk slot by
  more than the st double-buffer can hide.  Wo streams in during
  pairs 5-7.  Scores issue before drains within a chunk.
- Host marshals every HBM operand into [128, X] partition-contiguous
  layout, split into ~256KB dma_starts: one DMA queue only sustains
  ~60-90 GB/s, so big tensors must spread across queues.
- LN epilogue: bn_stats/aggr + sqrt + small reciprocal; yt on ACT
  (idle by then), gamma/beta tensor_tensor on GPSIMD (else the tail
  is DVE-bound).
"""

import sys

if "/opt/trn_rl_repo" not in sys.path:
    sys.path.insert(0, "/opt/trn_rl_repo")

import numpy as np

import concourse.bacc as bacc
import concourse.bass as bass
import concourse.mybir as mybir
import concourse.tile as tile
from concourse.bass import ds, ts
from concourse.bass_utils import run_bass_kernel_spmd

BF16 = mybir.dt.bfloat16
FP32 = mybir.dt.float32
FP8 = mybir.dt.float8e4
AF = mybir.ActivationFunctionType
ALU = mybir.AluOpType
DR = mybir.MatmulPerfMode.DoubleRow

N_CORES = 8
B = 2
S = 2048
D = 1024
H = 16
P = 128

SL = S // 4          # 512 query rows per core
KC = D // P          # 8 k-tiles of 128
KH = KC // 2         # 4 DoubleRow k-groups
SQ = SL // P         # 4
CH = S // P          # 16 key chunks
PAIRS = H // 2       # 8 head pairs
NB = 4               # 4 key/value blocks of 512
EPS = 1e-5
WS = 32.0            # host weight scale (fp8 range)
EXP_SCALE = 0.125 / (WS * WS)
EXP_BIAS = -3.5

_NC_CACHE = {}


def build_nc():
    nc = bacc.Bacc(num_devices=N_CORES)

    # marshaled [128, X] partition-contiguous operands
    xqt_d = nc.dram_tensor("xqt", [P, KC, SL], BF16, kind="ExternalInput")
    xkt_d = nc.dram_tensor("xkt", [P, NB, KC, SL], BF16, kind="ExternalInput")
    xvt_d = nc.dram_tensor("xvt", [P, NB, KC, SL], FP8, kind="ExternalInput")
    xqres_d = nc.dram_tensor("xqres", [P, SQ, D], BF16, kind="ExternalInput")
    wq_d = nc.dram_tensor("wq", [P, KC, D], BF16, kind="ExternalInput")
    wk_d = nc.dram_tensor("wk", [P, KC, D], BF16, kind="ExternalInput")
    wv_d = nc.dram_tensor("wv", [P, KC, D], FP8, kind="ExternalInput")
    wo_d = nc.dram_tensor("wo", [P, KC, D], FP8, kind="ExternalInput")
    bq_d = nc.dram_tensor("bq", [P, KC], FP32, kind="ExternalInput")
    bk_d = nc.dram_tensor("bk", [P, KC], FP32, kind="ExternalInput")
    bv_d = nc.dram_tensor("bv", [D], FP32, kind="ExternalInput")
    gam_d = nc.dram_tensor("gam", [D], FP32, kind="ExternalInput")
    bet_d = nc.dram_tensor("bet", [D], FP32, kind="ExternalInput")
    ident_d = nc.dram_tensor("ident", [P, P], BF16, kind="ExternalInput")
    ones_d = nc.dram_tensor("ones", [P, 64], BF16, kind="ExternalInput")

    y_d = nc.dram_tensor("y", [SL, D], FP32, kind="ExternalOutput")

    with tile.TileContext(nc) as tc:
        with (
            tc.tile_pool(name="consts", bufs=1) as consts,
            tc.tile_pool(name="persist", bufs=1) as persist,
            tc.tile_pool(name="small", bufs=4) as small,
            tc.tile_pool(name="wbf", bufs=2) as wbf,
            tc.tile_pool(name="w8", bufs=2) as w8,
            tc.tile_pool(name="etp", bufs=3) as etp,
            tc.tile_pool(name="normp", bufs=2) as normp,
            tc.tile_pool(name="rowp", bufs=2) as rowp,
        ):
            # ---- persistent SBUF state ----
            kt_full = persist.tile([P, KC, S], BF16, tag="ktf")
            vf_full = persist.tile([P, NB, SQ, PAIRS, 130], FP8, tag="vff")
            qt_sb = persist.tile([P, KC, SL], BF16, tag="qt")
            ctx_sb = persist.tile([P, PAIRS, SL], FP8, tag="ctx")

            norm_pend = [None]

            def emit_normalize(p_, ut, bc_alloc):
                # ut: SBUF copy [P, 2, SL] bf16 of the U accumulators;
                # row 64 = raw softmax denominators Z[j, q].
                # Reshape the Z rows onto 128 partitions, reciprocal
                # there (fast), reshape back, then broadcast via PE.
                zrow = rowp.tile([P, 8], BF16, tag="zr")
                nc.sync.dma_start(zrow[:], ut[64:65, :, :])
                rcp = rowp.tile([P, 8], BF16, tag="rc")
                with nc.allow_low_precision(reason="softmax denom recip"):
                    nc.vector.reciprocal(out=rcp[:], in_=zrow[:])
                rrow = rowp.tile([P, 2, SL], BF16, tag="rr")
                nc.sync.dma_start(rrow[0:1, :, :], rcp[:])
                for j in range(2):
                    bc = bc_alloc()
                    nc.tensor.matmul(
                        bc[0:64, :],
                        ones64[0:1, :],
                        rrow[0:1, j, :],
                        start=True,
                        stop=True,
                    )
                    with nc.allow_low_precision(reason="ctx fp8"):
                        if j == 0:
                            nc.vector.tensor_tensor(
                                ctx_sb[0:64, p_, :],
                                ut[0:64, j, :],
                                bc[0:64, :],
                                ALU.mult,
                            )
                        else:
                            ctmp = normp.tile([P, SL], FP8, tag="ctmp")
                            nc.vector.tensor_tensor(
                                ctmp[0:64, :], ut[0:64, j, :], bc[0:64, :], ALU.mult
                            )
                            nc.sync.dma_start(ctx_sb[64:128, p_, :], ctmp[0:64, :])

            def run_pair(p, psSt, psU, bc_alloc, per_chunk=None):
                utA = psU.tile([P, SL], FP32, tag="ut")
                utB = psU.tile([P, SL], FP32, tag="ut")
                ets = {}
                for idx in range(CH + 2):
                    if idx < CH:
                        c = idx
                        g = c // 2
                        if c % 2 == 0:
                            ets[g] = etp.tile(
                                [P, 2, 2, SL], FP8, tag="et", name="et"
                            )
                        st = psSt.tile([P, 2, SL], FP32, tag="st")
                        ktt = kt_full[:, p, ds(c * P, P)]
                        nc.tensor.matmul(
                            st[:, 0, :],
                            ktt[0:64, :],
                            qt_sb[0:64, p, :],
                            start=True,
                            stop=True,
                        )
                        nc.tensor.matmul(
                            st[:, 1, :],
                            ktt[64:128, :],
                            qt_sb[64:128, p, :],
                            start=True,
                            stop=True,
                        )
                        with nc.allow_low_precision(reason="exp to fp8"):
                            nc.scalar.activation(
                                out=ets[g][:, c % 2, :, :],
                                in_=st[:],
                                func=AF.Exp,
                                scale=EXP_SCALE,
                                bias=ebias_t[:],
                            )
                        if per_chunk is not None:
                            per_chunk(c)
                    if idx == 1 and norm_pend[0] is not None:
                        pj, putc = norm_pend[0]
                        emit_normalize(pj, putc, bc_alloc)
                        norm_pend[0] = None
                    if idx >= 3 and (idx - 3) % 2 == 0:
                        g0 = (idx - 3) // 2
                        et0 = ets.pop(g0)
                        c0 = 2 * g0  # chunks c0, c0+1; same block, i even
                        blk, i = c0 // SQ, c0 % SQ
                        for j, ut in enumerate((utA, utB)):
                            nc.tensor.matmul(
                                ut[:65, :],
                                vf_full[:, blk, ds(i, 2), p, ds(j * 65, 65)],
                                et0[:, :, j, :],
                                start=(g0 == 0),
                                stop=(g0 == CH // 2 - 1),
                                perf_mode=DR,
                            )
                # free the PSUM banks: copy U to SBUF
                utc = normp.tile([P, 2, SL], BF16, tag="utc")
                nc.vector.tensor_copy(utc[0:65, 0, :], utA[0:65, :])
                nc.vector.tensor_copy(utc[0:65, 1, :], utB[0:65, :])
                norm_pend[0] = (p, utc)

            # Q-path DMAs only; everything else queues after these so the
            # first matmul starts as early as possible.
            def dma_bykt(dst, srcd, nk=KC):
                # one dma_start per k-tile: spreads a big tensor across
                # DMA queues (a single queue sustains only ~60-90 GB/s)
                for k in range(nk):
                    nc.sync.dma_start(dst[:, k], srcd[:, k])

            wq_sb = wbf.tile([P, KC, D], BF16, tag="w", name="wq")
            dma_bykt(wq_sb, wq_d)
            bq_sb = consts.tile([P, KC], FP32)
            nc.sync.dma_start(bq_sb[:], bq_d[:])
            wk_sb = wbf.tile([P, KC, D], BF16, tag="w", name="wk")
            bk_sb = consts.tile([P, KC], FP32)
            ident = consts.tile([P, P], BF16)
            ones64 = consts.tile([P, 64], BF16)
            eps_t = consts.tile([P, 1], FP32)
            ebias_t = consts.tile([P, 1], FP32)

            with (
                tc.tile_pool(name="xkp", bufs=1) as xkp,
                tc.tile_pool(name="xqp", bufs=1) as xqp,
                tc.tile_pool(name="xpool", bufs=4) as xpool,
                tc.tile_pool(name="psP", bufs=2, space="PSUM") as psP,
                tc.tile_pool(name="psSt", bufs=2, space="PSUM") as psSt,
                tc.tile_pool(name="psU", bufs=2, space="PSUM") as psU,
            ):
                xq_sb = xqp.tile([P, KC, SL], BF16, tag="xq", name="xq")
                dma_bykt(xq_sb, xqt_d)
                xk_sb = xkp.tile([P, NB, KC, SL], BF16, tag="xk")

                def emit_q(m):
                    pp = psP.tile([P, SL], FP32, tag="pp")
                    for k in range(KC):
                        nc.tensor.matmul(
                            pp[:],
                            wq_sb[:, k, ts(m, P)],
                            xq_sb[:, k, :],
                            start=(k == 0),
                            stop=(k == KC - 1),
                        )
                    with nc.allow_low_precision(reason="qt bf16"):
                        nc.vector.tensor_scalar(
                            out=qt_sb[:, m, :],
                            in0=pp[:],
                            scalar1=bq_sb[:, m : m + 1],
                            scalar2=None,
                            op0=ALU.add,
                        )

                # remaining loads stream in behind the Q projection
                dma_bykt(wk_sb, wk_d)
                for blk in range(NB):
                    for kg in range(KH):
                        nc.sync.dma_start(
                            xk_sb[:, blk, ds(2 * kg, 2), :],
                            xkt_d[:, blk, ds(2 * kg, 2), :],
                        )
                nc.sync.dma_start(bk_sb[:], bk_d[:])
                nc.sync.dma_start(ident[:], ident_d[:])
                nc.sync.dma_start(ones64[:], ones_d[:])
                nc.vector.memset(eps_t[:], EPS)
                nc.vector.memset(ebias_t[:], EXP_BIAS)

                def bcast_load(src, tag):
                    t = consts.tile([P, D], BF16, tag=tag)
                    ap = bass.AP(tensor=src, offset=0, ap=[[0, P], [1, D]])
                    nc.gpsimd.dma_start(out=t[:], in_=ap)
                    return t

                bv_b = bcast_load(bv_d, "bv_b")
                gam_b = bcast_load(gam_d, "gam_b")
                bet_b = bcast_load(bet_d, "bet_b")

                nc.vector.memset(vf_full[:, :, :, :, 64:65], 1.0)
                nc.vector.memset(vf_full[:, :, :, :, 129:130], 1.0)

                def emit_k(m, blk):
                    pp = psP.tile([P, SL], FP32, tag="pp")
                    for k in range(KC):
                        nc.tensor.matmul(
                            pp[:],
                            wk_sb[:, k, ts(m, P)],
                            xk_sb[:, blk, k, :],
                            start=(k == 0),
                            stop=(k == KC - 1),
                        )
                    with nc.allow_low_precision(reason="kt bf16"):
                        nc.vector.tensor_scalar(
                            out=kt_full[:, m, ds(blk * SL, SL)],
                            in0=pp[:],
                            scalar1=bk_sb[:, m : m + 1],
                            scalar2=None,
                            op0=ALU.add,
                        )

                wv_sb = w8.tile([P, KC, D], FP8, tag="w8", name="wv")
                for kg in range(KH):
                    nc.sync.dma_start(
                        wv_sb[:, ds(2 * kg, 2), :], wv_d[:, ds(2 * kg, 2), :]
                    )

                xv_sbs = {}

                def load_xv(blk):
                    x = xpool.tile([P, KC, SL], FP8, tag="x", name=f"xv{blk}")
                    for kg in range(KH):
                        nc.sync.dma_start(
                            x[:, ds(2 * kg, 2), :],
                            xvt_d[:, blk, ds(2 * kg, 2), :],
                        )
                    xv_sbs[blk] = x

                def emit_v(n, blk, i):
                    xv = xv_sbs[blk]
                    pp = psP.tile([P, 512], FP32, tag="pp")
                    for k in range(KH):
                        nc.tensor.matmul(
                            pp[:],
                            xv[:, ds(2 * k, 2), ts(i, P)],
                            wv_sb[:, ds(2 * k, 2), ds(n * 512, 512)],
                            start=(k == 0),
                            stop=(k == KH - 1),
                            perf_mode=DR,
                        )
                    vdst = vf_full[:, blk, i, ds(n * 4, 4), :].rearrange(
                        "q pl (j e) -> q pl j e", e=65
                    )
                    with nc.allow_low_precision(reason="vf fp8"):
                        nc.vector.tensor_tensor(
                            vdst[:, :, :, 0:64],
                            pp[:].rearrange("q (pl j e) -> q pl j e", pl=4, j=2),
                            bv_b[:, ds(n * 512, 512)].rearrange(
                                "q (pl j e) -> q pl j e", pl=4, j=2
                            ),
                            ALU.add,
                        )

                load_xv(0)
                load_xv(1)
                # prologue: just enough for pair 0 + pair 1 start
                emit_q(0)
                for blk in range(NB):
                    emit_k(0, blk)
                emit_q(1)
                emit_k(1, 0)

                # Deadline-ordered backlog.  kt m must land before pair m
                # consumes it (pair m starts at window chunk 16(m-1));
                # V n=1 lands during pair 4, just ahead of its own AVs.
                backlog = []
                for blk in range(1, NB):
                    backlog.append(("k", 1, blk))
                for m in range(2, 6):
                    backlog.append(("q", m, 0))
                    for blk in range(NB):
                        backlog.append(("k", m, blk))
                backlog_v1 = [("v1", blk, i) for blk in range(NB) for i in range(SQ)]
                backlog_late = []
                for m in range(6, KC):
                    backlog_late.append(("q", m, 0))
                    for blk in range(NB):
                        backlog_late.append(("k", m, blk))

                def drain(q):
                    if q:
                        kind, a, b_ = q.pop(0)
                        if kind == "k":
                            emit_k(a, b_)
                        elif kind == "q":
                            emit_q(a)
                        else:
                            emit_v(1, a, b_)

                def bc_alloc_a():
                    return psP.tile([P, 512], FP32, tag="pp", name="bca")

                # ---- all 8 pairs: projection work interleaved ----
                # pair 0: V n=0 just-in-time; pairs 1-3: K/Q every other
                # chunk; pair 4: V n=1 every chunk; pairs 5-6: the rest
                # every other chunk; pair 7: clean (safety drains only).
                def per_chunk(p, c):
                    if p == 0:
                        emit_v(0, c // SQ, c % SQ)
                        if c == 5:
                            load_xv(2)
                        if c == 9:
                            load_xv(3)
                    elif p in (1, 2, 3):
                        if c % 2 == 0:
                            drain(backlog)
                    elif p == 4:
                        drain(backlog_v1)
                        if backlog:
                            drain(backlog)
                    elif p in (5, 6):
                        if c % 2 == 0:
                            drain(backlog_late)
                    else:
                        drain(backlog)
                        drain(backlog_v1)
                        drain(backlog_late)

                wo_sb = None
                for p in range(PAIRS):
                    run_pair(
                        p, psSt, psU, bc_alloc_a, lambda c, p=p: per_chunk(p, c)
                    )
                    if p == 4:
                        # wv fully consumed; stream Wo in behind pairs 5-7
                        wo_sb = w8.tile([P, KC, D], FP8, tag="w8", name="wo")
                        for kg in range(KH):
                            nc.sync.dma_start(
                                wo_sb[:, ds(2 * kg, 2), :],
                                wo_d[:, ds(2 * kg, 2), :],
                            )

            # ------------- output projection + residual + LN -------------
            with (
                tc.tile_pool(name="outp", bufs=2) as outp,
                tc.tile_pool(name="psO", bufs=3, space="PSUM") as psO,
            ):

                def bc_alloc_o():
                    return psO.tile([P, 512], FP32, tag="pp", name="bco")

                pj, putc = norm_pend[0]
                emit_normalize(pj, putc, bc_alloc_o)
                norm_pend[0] = None

                for i in range(SQ):
                    res = outp.tile([P, D], BF16, tag="res")
                    nc.sync.dma_start(res[:], xqres_d[:, i, :])
                    pps = []
                    for n in range(2):
                        pp = psO.tile([P, 512], FP32, tag="pp")
                        # pairs 0-5 DoubleRow; 6 single; identity
                        # (1024*I); pair 7 last (its ctx lands latest)
                        for g in range(3):
                            nc.tensor.matmul(
                                pp[:],
                                ctx_sb[:, ds(2 * g, 2), ts(i, P)],
                                wo_sb[:, ds(2 * g, 2), ds(n * 512, 512)],
                                start=(g == 0),
                                stop=False,
                                perf_mode=DR,
                            )
                        nc.tensor.matmul(
                            pp[:],
                            ctx_sb[:, 6, ts(i, P)],
                            wo_sb[:, 6, ds(n * 512, 512)],
                            start=False,
                            stop=False,
                        )
                        nc.tensor.matmul(
                            pp[:],
                            ident[:],
                            res[:, ds(n * 512, 512)],
                            start=False,
                            stop=False,
                        )
                        nc.tensor.matmul(
                            pp[:],
                            ctx_sb[:, PAIRS - 1, ts(i, P)],
                            wo_sb[:, PAIRS - 1, ds(n * 512, 512)],
                            start=False,
                            stop=True,
                        )
                        pps.append(pp)
                    stats = small.tile([P, 2, 6], FP32, tag="stats")
                    nc.vector.bn_stats(stats[:, 0, :], pps[0][:])
                    nc.vector.bn_stats(stats[:, 1, :], pps[1][:])
                    mv = small.tile([P, 2], FP32, tag="mv")
                    nc.vector.bn_aggr(mv[:], stats[:])
                    std = small.tile([P, 1], FP32, tag="std")
                    nc.scalar.activation(
                        out=std[:],
                        in_=mv[:, 1:2],
                        func=AF.Sqrt,
                        bias=eps_t[:],
                        scale=1.0,
                    )
                    rstd = small.tile([P, 1], FP32, tag="rstd")
                    nc.vector.reciprocal(out=rstd[:], in_=std[:])
                    nmrs = small.tile([P, 1], FP32, tag="nmrs")
                    nc.vector.tensor_scalar(
                        out=nmrs[:],
                        in0=mv[:, 0:1],
                        scalar1=-1.0,
                        scalar2=None,
                        op0=ALU.mult,
                    )
                    nc.vector.tensor_tensor(nmrs[:], nmrs[:], rstd[:], ALU.mult)
                    yt = outp.tile([P, D], FP32, tag="yt")
                    for n in range(2):
                        nc.scalar.activation(
                            out=yt[:, ds(n * 512, 512)],
                            in_=pps[n][:],
                            func=AF.Identity,
                            bias=nmrs[:],
                            scale=rstd[:],
                        )
                    nc.gpsimd.tensor_tensor(yt[:], yt[:], gam_b[:], ALU.mult)
                    nc.gpsimd.tensor_tensor(yt[:], yt[:], bet_b[:], ALU.add)
                    nc.sync.dma_start(y_d[ts(i, P), :], yt[:])

    nc.compile()
    return nc


def get_nc():
    if "nc" not in _NC_CACHE:
        _NC_CACHE["nc"] = build_nc()
    return _NC_CACHE["nc"]


def kernel(
    query,
    key,
    value,
    Wq,
    bq,
    Wk,
    bk,
    Wv,
    bv,
    Wo,
    bo,
    ln_gamma,
    ln_beta,
    _trace=False,
    _trace_cores=None,
):
    import ml_dtypes

    bf16 = ml_dtypes.bfloat16
    f8 = ml_dtypes.float8_e4m3fn

    def to_bf(x):
        return np.ascontiguousarray(np.asarray(x, np.float32).astype(bf16))

    def to_f8(x):
        return np.ascontiguousarray(
            np.clip(np.asarray(x, np.float32), -240.0, 240.0).astype(f8)
        )

    def marshal_w(w, cast):
        # [D, D] -> [128, KC, D]: partition p, ktile k = row k*128+p
        return cast((np.asarray(w, np.float32) * WS).reshape(KC, P, D)
                    .transpose(1, 0, 2))

    def marshal_xt(xt, nblk, cast):
        # xt: [D, S'] (already transposed) -> [128, nblk, KC, 512]
        sp = xt.shape[1]
        blkw = sp // nblk
        r = xt.reshape(KC, P, nblk, blkw).transpose(1, 2, 0, 3)
        return cast(r)

    def marshal_b(b):
        return np.ascontiguousarray(
            (np.asarray(b, np.float32) * WS).reshape(KC, P).T
        )

    query = np.asarray(query, np.float32)
    key = np.asarray(key, np.float32)
    value = np.asarray(value, np.float32)
    bo = np.asarray(bo, np.float32)

    shared = {
        "wq": marshal_w(Wq, to_bf),
        "wk": marshal_w(Wk, to_bf),
        "wv": marshal_w(Wv, to_f8),
        "wo": marshal_w(Wo, to_f8),
        "bq": marshal_b(bq),
        "bk": marshal_b(bk),
        "bv": np.ascontiguousarray(np.asarray(bv, np.float32) * WS),
        "gam": np.ascontiguousarray(np.asarray(ln_gamma, np.float32)),
        "bet": np.ascontiguousarray(np.asarray(ln_beta, np.float32)),
        "ident": (np.eye(P, dtype=np.float32) * (WS * WS)).astype(bf16),
        "ones": np.ones((P, 64), dtype=np.float32).astype(bf16),
    }
    in_maps = []
    for c in range(N_CORES):
        b, r = divmod(c, NB)
        rows = slice(r * SL, (r + 1) * SL)
        m = dict(shared)
        m["xqt"] = marshal_xt(query[b, rows, :].T, 1, to_bf).reshape(P, KC, SL)
        m["xkt"] = marshal_xt(key[b].T, NB, to_bf)
        m["xvt"] = marshal_xt(value[b].T, NB, to_f8)
        # [SL, D] -> [128, SQ, D]
        m["xqres"] = to_bf(
            (query[b, rows, :] + bo[None, :]).reshape(SQ, P, D).transpose(1, 0, 2)
        )
        in_maps.append(m)

    nc = get_nc()
    res = run_bass_kernel_spmd(
        nc,
        in_maps,
        list(range(N_CORES)),
        trace=_trace,
        trace_cores=_trace_cores,
    )
    out = np.empty((B, S, D), dtype=np.float32)
    for c in range(N_CORES):
        b, r = divmod(c, NB)
        out[b, r * SL : (r + 1) * SL, :] = res.results[c]["y"]
    if _trace:
        return out, res
    return out


# revision 3
# speedup vs baseline: 1.0008x; 1.0008x over previous
"""Multi-head attention + residual + LayerNorm on 8 Trainium2 NeuronCores, v6.

Sharding: core c handles batch b = c//4 and query-row quarter r = c%4
(rows 512r..512r+512 of S=2048) with ALL 16 heads.  K/V projected
redundantly per batch group (collectives ~110us/2MB on this stack),
projection work interleaved into the ACT-bound attention chunk stream.

v6 over v4 (mixed fp8/bf16 + DoubleRow where the math tolerates it):
- Q/K path stays bf16 (weights host-scaled x32): softmax exponentiates
  score errors, and fp8 Q/K measured 2.5e-2 rel err (over tolerance).
- V and O projections and the AV matmul run fp8e4 DoubleRow (2 k-tiles
  per instruction): V lands fp8 at x32 scale, ctx fp8 at x32, Wo fp8
  x32; the x1024 on the O-projection PSUM is matched by a 1024*I
  residual identity, and LayerNorm is scale-invariant (rstd absorbs
  it, eps 1e-5 is negligible at either scale for this data).
- exp runs with scale 0.125/1024 and bias -3.5 so et fits fp8e4: max
  scaled score on this input set is 8.29, and TRN fp8e4 becomes Inf
  above 240 (et_max = e^4.79 = 120, 2x margin; bias cancels in
  softmax).  AV consumes et over chunk pairs [P, 2, 2, SL].
- Softmax denominators: reciprocal of a [1,1024] row was 3.2us on DVE
  (big-tile reciprocal is ~20x slower than copy); now the row is
  DMA-reshaped to [128,8], reciprocal'd there (~0.2us), DMA'd back,
  then PE-broadcast (ones matmul) as before.
- Q/K projection PSUM->SBUF copies moved from ACT to DVE
  (tensor_scalar add with per-partition bias AP): ACT does (almost)
  nothing but the 128 exps at (1024+352)/1.2 = 1147ns each, which are
  the kernel's critical path and lower bound (~147us).
- Projection backlog is deadline-ordered and paced: K/Q items (1.7us
  each) drain every other chunk in pairs 1-3 and 5-6, V n=1 (0.85us)
  every chunk of pair 4, so PE never oversubscribes a chunk slot by
  more than the st double-buffer can hide.  Wo streams in during
  pairs 5-7.  Scores issue before drains within a chunk.
- Host marshals every HBM operand into [128, X] partition-contiguous
  layout, split into ~256KB dma_starts: one DMA queue only sustains
  ~60-90 GB/s, so big tensors must spread across queues.
- LN epilogue: bn_stats/aggr + sqrt + small reciprocal; yt on ACT
  (idle by then), gamma/beta tensor_tensor on GPSIMD (else the tail
  is DVE-bound).
"""

import sys

if "/opt/trn_rl_repo" not in sys.path:
    sys.path.insert(0, "/opt/trn_rl_repo")

import numpy as np

import concourse.bacc as bacc
import concourse.bass as bass
import concourse.mybir as mybir
import concourse.tile as tile
from concourse.bass import ds, ts
from concourse.bass_utils import run_bass_kernel_spmd

BF16 = mybir.dt.bfloat16
FP32 = mybir.dt.float32
FP8 = mybir.dt.float8e4
AF = mybir.ActivationFunctionType
ALU = mybir.AluOpType
DR = mybir.MatmulPerfMode.DoubleRow

N_CORES = 8
B = 2
S = 2048
D = 1024
H = 16
P = 128

SL = S // 4          # 512 query rows per core
KC = D // P          # 8 k-tiles of 128
KH = KC // 2         # 4 DoubleRow k-groups
SQ = SL // P         # 4
CH = S // P          # 16 key chunks
PAIRS = H // 2       # 8 head pairs
NB = 4               # 4 key/value blocks of 512
EPS = 1e-5
WS = 32.0            # host weight scale (fp8 range)
EXP_SCALE = 0.125 / (WS * WS)
EXP_BIAS = -3.5

_NC_CACHE = {}


def build_nc():
    nc = bacc.Bacc(num_devices=N_CORES)

    # marshaled [128, X] partition-contiguous operands
    xqt_d = nc.dram_tensor("xqt", [P, KC, SL], BF16, kind="ExternalInput")
    xkt_d = nc.dram_tensor("xkt", [P, NB, KC, SL], BF16, kind="ExternalInput")
    xvt_d = nc.dram_tensor("xvt", [P, NB, KC, SL], FP8, kind="ExternalInput")
    xqres_d = nc.dram_tensor("xqres", [P, SQ, D], BF16, kind="ExternalInput")
    wq_d = nc.dram_tensor("wq", [P, KC, KC, P], BF16, kind="ExternalInput")
    wk_d = nc.dram_tensor("wk", [P, KC, KC, P], BF16, kind="ExternalInput")
    wv_d = nc.dram_tensor("wv", [P, KC, D], FP8, kind="ExternalInput")
    wo_d = nc.dram_tensor("wo", [P, KC, D], FP8, kind="ExternalInput")
    bq_d = nc.dram_tensor("bq", [P, KC], FP32, kind="ExternalInput")
    bk_d = nc.dram_tensor("bk", [P, KC], FP32, kind="ExternalInput")
    bv_d = nc.dram_tensor("bv", [D], FP32, kind="ExternalInput")
    gam_d = nc.dram_tensor("gam", [D], FP32, kind="ExternalInput")
    bet_d = nc.dram_tensor("bet", [D], FP32, kind="ExternalInput")
    ident_d = nc.dram_tensor("ident", [P, P], BF16, kind="ExternalInput")
    ones_d = nc.dram_tensor("ones", [P, 64], BF16, kind="ExternalInput")

    y_d = nc.dram_tensor("y", [SL, D], FP32, kind="ExternalOutput")

    with tile.TileContext(nc) as tc:
        with (
            tc.tile_pool(name="consts", bufs=1) as consts,
            tc.tile_pool(name="persist", bufs=1) as persist,
            tc.tile_pool(name="small", bufs=4) as small,
            tc.tile_pool(name="wbf", bufs=2) as wbf,
            tc.tile_pool(name="w8", bufs=2) as w8,
            tc.tile_pool(name="etp", bufs=3) as etp,
            tc.tile_pool(name="normp", bufs=2) as normp,
            tc.tile_pool(name="rowp", bufs=2) as rowp,
        ):
            # ---- persistent SBUF state ----
            kt_full = persist.tile([P, KC, S], BF16, tag="ktf")
            vf_full = persist.tile([P, NB, SQ, PAIRS, 130], FP8, tag="vff")
            qt_sb = persist.tile([P, KC, SL], BF16, tag="qt")
            ctx_sb = persist.tile([P, PAIRS, SL], FP8, tag="ctx")
            ctx7 = persist.tile([P, SL], FP8, tag="ctx7")

            norm_pend = [None]

            def emit_normalize(p_, ut, bc_alloc):
                # ut: SBUF copy [P, 2, SL] bf16 of the U accumulators;
                # row 64 = raw softmax denominators Z[j, q].
                # Reshape the Z rows onto 128 partitions, reciprocal
                # there (fast), reshape back, then broadcast via PE.
                zrow = rowp.tile([P, 8], BF16, tag="zr")
                nc.sync.dma_start(zrow[:], ut[64:65, :, :])
                rcp = rowp.tile([P, 8], BF16, tag="rc")
                with nc.allow_low_precision(reason="softmax denom recip"):
                    nc.vector.reciprocal(out=rcp[:], in_=zrow[:])
                rrow = rowp.tile([P, 2, SL], BF16, tag="rr")
                nc.sync.dma_start(rrow[0:1, :, :], rcp[:])
                cdst = ctx7 if p_ == PAIRS - 1 else ctx_sb[:, p_]
                for j in range(2):
                    bc = bc_alloc()
                    nc.tensor.matmul(
                        bc[0:64, :],
                        ones64[0:1, :],
                        rrow[0:1, j, :],
                        start=True,
                        stop=True,
                    )
                    with nc.allow_low_precision(reason="ctx fp8"):
                        if j == 0:
                            nc.vector.tensor_tensor(
                                cdst[0:64, :],
                                ut[0:64, j, :],
                                bc[0:64, :],
                                ALU.mult,
                            )
                        else:
                            ctmp = normp.tile([P, SL], FP8, tag="ctmp")
                            nc.vector.tensor_tensor(
                                ctmp[0:64, :], ut[0:64, j, :], bc[0:64, :], ALU.mult
                            )
                            nc.sync.dma_start(cdst[64:128, :], ctmp[0:64, :])

            def run_pair(p, psSt, psU, bc_alloc, per_chunk=None):
                utA = psU.tile([P, SL], FP32, tag="ut")
                utB = psU.tile([P, SL], FP32, tag="ut")
                ets = {}
                for idx in range(CH + 2):
                    if idx < CH:
                        c = idx
                        if per_chunk is not None:
                            per_chunk(c)
                        g = c // 2
                        if c % 2 == 0:
                            ets[g] = etp.tile(
                                [P, 2, 2, SL], FP8, tag="et", name="et"
                            )
                        st = psSt.tile([P, 2, SL], FP32, tag="st")
                        ktt = kt_full[:, p, ds(c * P, P)]
                        nc.tensor.matmul(
                            st[:, 0, :],
                            ktt[0:64, :],
                            qt_sb[0:64, p, :],
                            start=True,
                            stop=True,
                        )
                        nc.tensor.matmul(
                            st[:, 1, :],
                            ktt[64:128, :],
                            qt_sb[64:128, p, :],
                            start=True,
                            stop=True,
                        )
                        with nc.allow_low_precision(reason="exp to fp8"):
                            nc.scalar.activation(
                                out=ets[g][:, c % 2, :, :],
                                in_=st[:],
                                func=AF.Exp,
                                scale=EXP_SCALE,
                                bias=ebias_t[:],
                            )
                    if idx == 1 and norm_pend[0] is not None:
                        pj, putc = norm_pend[0]
                        emit_normalize(pj, putc, bc_alloc)
                        norm_pend[0] = None
                    if idx >= 3 and (idx - 3) % 2 == 0:
                        g0 = (idx - 3) // 2
                        et0 = ets.pop(g0)
                        c0 = 2 * g0  # chunks c0, c0+1; same block, i even
                        blk, i = c0 // SQ, c0 % SQ
                        for j, ut in enumerate((utA, utB)):
                            nc.tensor.matmul(
                                ut[:65, :],
                                vf_full[:, blk, ds(i, 2), p, ds(j * 65, 65)],
                                et0[:, :, j, :],
                                start=(g0 == 0),
                                stop=(g0 == CH // 2 - 1),
                                perf_mode=DR,
                            )
                # free the PSUM banks: copy U to SBUF
                utc = normp.tile([P, 2, SL], BF16, tag="utc")
                nc.vector.tensor_copy(utc[0:65, 0, :], utA[0:65, :])
                nc.vector.tensor_copy(utc[0:65, 1, :], utB[0:65, :])
                norm_pend[0] = (p, utc)

            # Q-path DMAs only; everything else queues after these so the
            # first matmul starts as early as possible.
            def dma_bykt(dst, srcd, nk=KC):
                # one dma_start per leading tile: spreads a big tensor
                # across DMA queues / lets consumers start early
                for k in range(nk):
                    nc.sync.dma_start(dst[:, k], srcd[:, k])

            # weights land m-major: Q/K projection m needs only its own
            # 256KB chunk, so the first matmul starts much earlier
            wq_sb = wbf.tile([P, KC, KC, P], BF16, tag="w", name="wq")
            bq_sb = consts.tile([P, KC], FP32)
            wk_sb = wbf.tile([P, KC, KC, P], BF16, tag="w", name="wk")
            bk_sb = consts.tile([P, KC], FP32)
            ident = consts.tile([P, P], BF16)
            ones64 = consts.tile([P, 64], BF16)
            eps_t = consts.tile([P, 1], FP32)
            ebias_t = consts.tile([P, 1], FP32)

            with (
                tc.tile_pool(name="xkp", bufs=1) as xkp,
                tc.tile_pool(name="xqp", bufs=1) as xqp,
                tc.tile_pool(name="xpool", bufs=4) as xpool,
                tc.tile_pool(name="psP", bufs=2, space="PSUM") as psP,
                tc.tile_pool(name="psSt", bufs=2, space="PSUM") as psSt,
                tc.tile_pool(name="psU", bufs=2, space="PSUM") as psU,
            ):
                xq_sb = xqp.tile([P, KC, SL], BF16, tag="xq", name="xq")
                xk_sb_ = xkp.tile([P, NB, KC, SL], BF16, tag="xk", name="xk_sb_")
                # dependency-ordered loads with big per-partition lines
                # (4-8KB): Q m0 needs xq + wq[m0] only; K(0,blk0) needs
                # wk[m0] + xk[blk0].  One dma_start spreads its 128
                # partition rows across all 16 queues.
                nc.sync.dma_start(xq_sb[:], xqt_d[:])
                nc.sync.dma_start(wq_sb[:, 0], wq_d[:, 0])
                nc.sync.dma_start(bq_sb[:], bq_d[:])
                nc.sync.dma_start(wk_sb[:, 0], wk_d[:, 0])
                nc.sync.dma_start(xk_sb_[:, 0, :, :], xkt_d[:, 0, :, :])
                for m in range(1, KC):
                    nc.sync.dma_start(wq_sb[:, m], wq_d[:, m])
                    nc.sync.dma_start(wk_sb[:, m], wk_d[:, m])
                proj_pend = {}
                xk_sb = xk_sb_

                def emit_q(m, h=None, on_act=False):
                    hs = range(KC) if h is None else range(h * KH, (h + 1) * KH)
                    if h in (None, 0):
                        proj_pend["q", m] = psP.tile([P, SL], FP32, tag="pp", name="ppq")
                    pp = proj_pend["q", m]
                    for k in hs:
                        nc.tensor.matmul(
                            pp[:],
                            wq_sb[:, m, k, :],
                            xq_sb[:, k, :],
                            start=(k == 0),
                            stop=(k == KC - 1),
                        )
                    if h == 0:
                        return
                    del proj_pend["q", m]
                    if on_act:
                        nc.scalar.activation(
                            out=qt_sb[:, m, :],
                            in_=pp[:],
                            func=AF.Identity,
                            bias=bq_sb[:, m : m + 1],
                        )
                        return
                    with nc.allow_low_precision(reason="qt bf16"):
                        nc.vector.tensor_scalar(
                            out=qt_sb[:, m, :],
                            in0=pp[:],
                            scalar1=bq_sb[:, m : m + 1],
                            scalar2=None,
                            op0=ALU.add,
                        )

                # remaining loads stream in behind the Q projection
                for blk in range(1, NB):
                    nc.sync.dma_start(xk_sb[:, blk, :, :], xkt_d[:, blk, :, :])
                nc.sync.dma_start(bk_sb[:], bk_d[:])
                nc.sync.dma_start(ident[:], ident_d[:])
                nc.sync.dma_start(ones64[:], ones_d[:])
                nc.vector.memset(eps_t[:], EPS)
                nc.vector.memset(ebias_t[:], EXP_BIAS)

                def bcast_load(src, tag):
                    t = consts.tile([P, D], BF16, tag=tag)
                    ap = bass.AP(tensor=src, offset=0, ap=[[0, P], [1, D]])
                    nc.gpsimd.dma_start(out=t[:], in_=ap)
                    return t

                bv_b = bcast_load(bv_d, "bv_b")
                gam_b = bcast_load(gam_d, "gam_b")
                bet_b = bcast_load(bet_d, "bet_b")

                nc.vector.memset(vf_full[:, :, :, :, 64:65], 1.0)
                nc.vector.memset(vf_full[:, :, :, :, 129:130], 1.0)

                def emit_k(m, blk, h=None, on_act=False):
                    hs = range(KC) if h is None else range(h * KH, (h + 1) * KH)
                    if h in (None, 0):
                        proj_pend["k", m, blk] = psP.tile([P, SL], FP32, tag="pp", name="ppk")
                    pp = proj_pend["k", m, blk]
                    for k in hs:
                        nc.tensor.matmul(
                            pp[:],
                            wk_sb[:, m, k, :],
                            xk_sb[:, blk, k, :],
                            start=(k == 0),
                            stop=(k == KC - 1),
                        )
                    if h == 0:
                        return
                    del proj_pend["k", m, blk]
                    if on_act:
                        nc.scalar.activation(
                            out=kt_full[:, m, ds(blk * SL, SL)],
                            in_=pp[:],
                            func=AF.Identity,
                            bias=bk_sb[:, m : m + 1],
                        )
                        return
                    with nc.allow_low_precision(reason="kt bf16"):
                        nc.vector.tensor_scalar(
                            out=kt_full[:, m, ds(blk * SL, SL)],
                            in0=pp[:],
                            scalar1=bk_sb[:, m : m + 1],
                            scalar2=None,
                            op0=ALU.add,
                        )

                wv_sb = w8.tile([P, KC, D], FP8, tag="w8", name="wv")
                for kg in range(KH):
                    nc.sync.dma_start(
                        wv_sb[:, ds(2 * kg, 2), :], wv_d[:, ds(2 * kg, 2), :]
                    )

                xv_sbs = {}

                def load_xv(blk):
                    x = xpool.tile([P, KC, SL], FP8, tag="x", name=f"xv{blk}")
                    for kg in range(KH):
                        nc.sync.dma_start(
                            x[:, ds(2 * kg, 2), :],
                            xvt_d[:, blk, ds(2 * kg, 2), :],
                        )
                    xv_sbs[blk] = x

                def emit_v(n, blk, i):
                    xv = xv_sbs[blk]
                    pp = psP.tile([P, 512], FP32, tag="pp")
                    for k in range(KH):
                        nc.tensor.matmul(
                            pp[:],
                            xv[:, ds(2 * k, 2), ts(i, P)],
                            wv_sb[:, ds(2 * k, 2), ds(n * 512, 512)],
                            start=(k == 0),
                            stop=(k == KH - 1),
                            perf_mode=DR,
                        )
                    vdst = vf_full[:, blk, i, ds(n * 4, 4), :].rearrange(
                        "q pl (j e) -> q pl j e", e=65
                    )
                    with nc.allow_low_precision(reason="vf fp8"):
                        nc.vector.tensor_tensor(
                            vdst[:, :, :, 0:64],
                            pp[:].rearrange("q (pl j e) -> q pl j e", pl=4, j=2),
                            bv_b[:, ds(n * 512, 512)].rearrange(
                                "q (pl j e) -> q pl j e", pl=4, j=2
                            ),
                            ALU.add,
                        )

                load_xv(0)
                load_xv(1)
                # prologue: just enough for pair 0 + pair 1 start.
                # PSUM->SBUF copies ride ACT, which idles before exps.
                emit_q(0, on_act=True)
                emit_k(0, 0, on_act=True)
                emit_q(1, on_act=True)
                for blk in range(1, NB):
                    emit_k(0, blk, on_act=True)
                emit_k(1, 0, on_act=True)

                # Deadline-ordered backlog of projection half-items
                # (4 matmuls, ~850ns: fits a chunk's PE slack).  kt m
                # must land before pair m consumes it (pair m starts at
                # window chunk 16(m-1)); V n=1 lands during pair 4,
                # just ahead of its own AVs.
                def halves(items):
                    return [(k, a, b_, h) for (k, a, b_) in items for h in (0, 1)]

                pre = [("k", 1, blk) for blk in range(1, NB)]
                for m in range(2, 6):
                    pre.append(("q", m, 0))
                    pre += [("k", m, blk) for blk in range(NB)]
                backlog = halves(pre)
                backlog_v1 = [("v1", blk, i, None) for blk in range(NB) for i in range(SQ)]
                late = []
                for m in range(6, KC):
                    late.append(("q", m, 0))
                    late += [("k", m, blk) for blk in range(NB)]
                backlog_late = halves(late)

                def drain(q):
                    if q:
                        kind, a, b_, h = q.pop(0)
                        if kind == "k":
                            emit_k(a, b_, h)
                        elif kind == "q":
                            emit_q(a, h)
                        else:
                            emit_v(1, a, b_)

                def bc_alloc_a():
                    return psP.tile([P, 512], FP32, tag="pp", name="bca")

                # ---- all 8 pairs: projection work interleaved ----
                # pair 0: V n=0 just-in-time; pairs 1-3: K/Q halves
                # every chunk; pair 4: V n=1 every chunk; pairs 5-6:
                # the rest; pair 7: clean (safety drains only).
                def per_chunk(p, c):
                    if p == 0:
                        emit_v(0, c // SQ, c % SQ)
                        if c == 5:
                            load_xv(2)
                        if c == 9:
                            load_xv(3)
                    elif p in (1, 2, 3):
                        drain(backlog)
                    elif p == 4:
                        drain(backlog_v1)
                        if backlog:
                            drain(backlog)
                    elif p in (5, 6):
                        drain(backlog_late)
                    else:
                        drain(backlog)
                        drain(backlog_v1)
                        drain(backlog_late)

                wo_sb = None
                for p in range(PAIRS):
                    run_pair(
                        p, psSt, psU, bc_alloc_a, lambda c, p=p: per_chunk(p, c)
                    )
                    if p == 4:
                        # wv fully consumed; stream Wo in behind pairs 5-7
                        wo_sb = w8.tile([P, KC, D], FP8, tag="w8", name="wo")
                        for kg in range(KH):
                            nc.sync.dma_start(
                                wo_sb[:, ds(2 * kg, 2), :],
                                wo_d[:, ds(2 * kg, 2), :],
                            )

            # ------------- output projection + residual + LN -------------
            with (
                tc.tile_pool(name="outp", bufs=4) as outp,
                tc.tile_pool(name="psO", bufs=4, space="PSUM") as psO,
            ):

                def bc_alloc_o():
                    return psO.tile([P, 512], FP32, tag="pp", name="bco")

                res_tiles = []
                for i in range(SQ):
                    res = outp.tile([P, D], BF16, tag="res", name=f"res{i}")
                    nc.sync.dma_start(res[:], xqres_d[:, i, :])
                    res_tiles.append(res)

                pj, putc = norm_pend[0]
                emit_normalize(pj, putc, bc_alloc_o)
                norm_pend[0] = None

                for i in range(SQ):
                    res = res_tiles[i]
                    pps = []
                    for n in range(2):
                        pp = psO.tile([P, 512], FP32, tag="pp")
                        # pairs 0-5 DoubleRow; 6 single; identity
                        # (1024*I); pair 7 last (its ctx lands latest)
                        for g in range(3):
                            nc.tensor.matmul(
                                pp[:],
                                ctx_sb[:, ds(2 * g, 2), ts(i, P)],
                                wo_sb[:, ds(2 * g, 2), ds(n * 512, 512)],
                                start=(g == 0),
                                stop=False,
                                perf_mode=DR,
                            )
                        nc.tensor.matmul(
                            pp[:],
                            ctx_sb[:, 6, ts(i, P)],
                            wo_sb[:, 6, ds(n * 512, 512)],
                            start=False,
                            stop=False,
                        )
                        nc.tensor.matmul(
                            pp[:],
                            ident[:],
                            res[:, ds(n * 512, 512)],
                            start=False,
                            stop=False,
                        )
                        nc.tensor.matmul(
                            pp[:],
                            ctx7[:, ts(i, P)],
                            wo_sb[:, PAIRS - 1, ds(n * 512, 512)],
                            start=False,
                            stop=True,
                        )
                        pps.append(pp)
                    stats = small.tile([P, 2, 6], FP32, tag="stats")
                    nc.vector.bn_stats(stats[:, 0, :], pps[0][:])
                    nc.vector.bn_stats(stats[:, 1, :], pps[1][:])
                    mv = small.tile([P, 2], FP32, tag="mv")
                    nc.vector.bn_aggr(mv[:], stats[:])
                    std = small.tile([P, 1], FP32, tag="std")
                    nc.scalar.activation(
                        out=std[:],
                        in_=mv[:, 1:2],
                        func=AF.Sqrt,
                        bias=eps_t[:],
                        scale=1.0,
                    )
                    rstd = small.tile([P, 1], FP32, tag="rstd")
                    nc.vector.reciprocal(out=rstd[:], in_=std[:])
                    nmrs = small.tile([P, 1], FP32, tag="nmrs")
                    nc.vector.tensor_scalar(
                        out=nmrs[:],
                        in0=mv[:, 0:1],
                        scalar1=-1.0,
                        scalar2=None,
                        op0=ALU.mult,
                    )
                    nc.vector.tensor_tensor(nmrs[:], nmrs[:], rstd[:], ALU.mult)
                    yt = outp.tile([P, D], FP32, tag="yt")
                    for n in range(2):
                        nc.scalar.activation(
                            out=yt[:, ds(n * 512, 512)],
                            in_=pps[n][:],
                            func=AF.Identity,
                            bias=nmrs[:],
                            scale=rstd[:],
                        )
                    nc.gpsimd.tensor_tensor(yt[:], yt[:], gam_b[:], ALU.mult)
                    nc.gpsimd.tensor_tensor(yt[:], yt[:], bet_b[:], ALU.add)
                    nc.sync.dma_start(y_d[ts(i, P), :], yt[:])

    nc.compile()
    return nc


def get_nc():
    if "nc" not in _NC_CACHE:
        _NC_CACHE["nc"] = build_nc()
    return _NC_CACHE["nc"]


def kernel(
    query,
    key,
    value,
    Wq,
    bq,
    Wk,
    bk,
    Wv,
    bv,
    Wo,
    bo,
    ln_gamma,
    ln_beta,
    _trace=False,
    _trace_cores=None,
):
    import ml_dtypes

    bf16 = ml_dtypes.bfloat16
    f8 = ml_dtypes.float8_e4m3fn

    def to_bf(x):
        return np.ascontiguousarray(np.asarray(x, np.float32).astype(bf16))

    def to_f8(x):
        return np.ascontiguousarray(
            np.clip(np.asarray(x, np.float32), -240.0, 240.0).astype(f8)
        )

    def marshal_w(w, cast):
        # [D, D] -> [128, KC, D]: partition p, ktile k = row k*128+p
        return cast((np.asarray(w, np.float32) * WS).reshape(KC, P, D)
                    .transpose(1, 0, 2))

    def marshal_w_m(w, cast):
        # [D, D] -> [128, M, KC, 128] m-major: chunk m holds all k-tiles
        # of output columns m*128..(m+1)*128
        return cast((np.asarray(w, np.float32) * WS)
                    .reshape(KC, P, KC, P).transpose(1, 2, 0, 3))

    def marshal_xt(xt, nblk, cast):
        # xt: [D, S'] (already transposed) -> [128, nblk, KC, 512]
        sp = xt.shape[1]
        blkw = sp // nblk
        r = xt.reshape(KC, P, nblk, blkw).transpose(1, 2, 0, 3)
        return cast(r)

    def marshal_b(b):
        return np.ascontiguousarray(
            (np.asarray(b, np.float32) * WS).reshape(KC, P).T
        )

    query = np.asarray(query, np.float32)
    key = np.asarray(key, np.float32)
    value = np.asarray(value, np.float32)
    bo = np.asarray(bo, np.float32)

    shared = {
        "wq": marshal_w_m(Wq, to_bf),
        "wk": marshal_w_m(Wk, to_bf),
        "wv": marshal_w(Wv, to_f8),
        "wo": marshal_w(Wo, to_f8),
        "bq": marshal_b(bq),
        "bk": marshal_b(bk),
        "bv": np.ascontiguousarray(np.asarray(bv, np.float32) * WS),
        "gam": np.ascontiguousarray(np.asarray(ln_gamma, np.float32)),
        "bet": np.ascontiguousarray(np.asarray(ln_beta, np.float32)),
        "ident": (np.eye(P, dtype=np.float32) * (WS * WS)).astype(bf16),
        "ones": np.ones((P, 64), dtype=np.float32).astype(bf16),
    }
    in_maps = []
    for c in range(N_CORES):
        b, r = divmod(c, NB)
        rows = slice(r * SL, (r + 1) * SL)
        m = dict(shared)
        m["xqt"] = marshal_xt(query[b, rows, :].T, 1, to_bf).reshape(P, KC, SL)
        m["xkt"] = marshal_xt(key[b].T, NB, to_bf)
        m["xvt"] = marshal_xt(value[b].T, NB, to_f8)
        # [SL, D] -> [128, SQ, D]
        m["xqres"] = to_bf(
            (query[b, rows, :] + bo[None, :]).reshape(SQ, P, D).transpose(1, 0, 2)
        )
        in_maps.append(m)

    nc = get_nc()
    res = run_bass_kernel_spmd(
        nc,
        in_maps,
        list(range(N_CORES)),
        trace=_trace,
        trace_cores=_trace_cores,
    )
    out = np.empty((B, S, D), dtype=np.float32)
    for c in range(N_CORES):
        b, r = divmod(c, NB)
        out[b, r * SL : (r + 1) * SL, :] = res.results[c]["y"]
    if _trace:
        return out, res
    return out
